# revision 8
# baseline (speedup 1.0000x reference)
"""Trainium2 Bass kernel for nn_AttentionBlock (B=16, C=512, H=W=32, 4 heads).

Strategy: data-parallel over batch across 8 NeuronCores (2 images/core).
Everything on-chip is computed in "transposed" (feature-major) layout so that
no PE transposes are needed anywhere:

  per batch b (x[b] is (C, N) = already feature-major, N = H*W = 1024 tokens):
    qT, kT = per-head slices of (w_qk^T @ x[b])          # (128, N) each
    v      = x[b]^T @ w_v  computed as (N, 512) tiles     # token-major
    sT     = kT^T-contracted scores in (j, i) layout      # via lhsT=kT, rhs=qT
    eT     = exp(sT * scale)        (no max subtraction: |s*scale| <= ~7)
    D      = ones^T @ eT            (col sums, replicated on 128 partitions)
    resT   = (v^T-contracted PV) / D                      # (128d, N) per head
    y[b]   = w_out^T @ resT + b_eff + x[b]                # (C, N) feature-major

Softmax over j is exact (exp / sum over j); max-subtraction is omitted because
softmax is shift-invariant and the scaled scores are O(10) for any plausible
input scale (fp32 exp overflows only beyond ~88).
"""

import numpy as np

import concourse.bass as bass
import concourse.mybir as mybir
import concourse.tile as tile
from concourse import bacc
from concourse.bass_utils import run_bass_kernel_spmd

dt = mybir.dt

N_CORES = 8
B = 16
C = 512
HEADS = 4
DK = C // HEADS          # 128
N = 1024                 # H*W tokens
SCALE = float(DK) ** -0.5
BPC = B // N_CORES       # batches per core = 2
CT = C // 128            # 4 contraction tiles over channels
NB = N // 512            # 2 moving-dim blocks of 512 tokens
JT = N // 128            # 8 key-token tiles
GQK = 2 * HEADS          # 8 q/k feature tiles (g = 2h + t, t: 0=q 1=k)

# Matmul input dtype knob: float32 (exact, 4 PE cycles/row) or float32r
# (single-pass, 1 cycle/row at N>=256). Data stays fp32 bits in SBUF either
# way; float32r only relaxes the PE multiply precision. HW-measured on the
# full problem: f32r rel err 1.9e-4 at ~169us/core vs fp32 9.1e-7 at
# ~574us/core. Default f32r.
import os
MM_F32R = os.environ.get("MM_F32R", "1") == "1"

LAST_RESULTS = None  # BassKernelResults of the most recent run (for test.py)


MMDT = dt.float32r if MM_F32R else dt.float32


def _mm(ap):
    return ap


def build_program():
    nc = bacc.Bacc("TRN2", target_bir_lowering=False, debug=False,
                   num_devices=N_CORES)

    x = nc.dram_tensor("x", [BPC, C, N], dt.float32, kind="ExternalInput").ap()
    wqk = nc.dram_tensor("wqk", [C, GQK * DK], dt.float32, kind="ExternalInput").ap()
    wv = nc.dram_tensor("wv", [C, C], dt.float32, kind="ExternalInput").ap()
    wout = nc.dram_tensor("wout", [C, C], dt.float32, kind="ExternalInput").ap()
    bqk = nc.dram_tensor("bqk", [128, GQK], dt.float32, kind="ExternalInput").ap()
    beff = nc.dram_tensor("beff", [128, CT], dt.float32, kind="ExternalInput").ap()
    ones = nc.dram_tensor("ones", [128, 128], dt.float32, kind="ExternalInput").ap()
    y = nc.dram_tensor("y", [BPC, C, N], dt.float32, kind="ExternalOutput").ap()

    with tile.TileContext(nc) as tc:
        _B = lambda k, d: int(os.environ.get(k, d))
        with (
            tc.tile_pool(name="weights", bufs=1) as wpool,
            tc.tile_pool(name="xin", bufs=2) as xpool,
            tc.tile_pool(name="qk", bufs=_B("QK_BUFS", 1)) as qkpool,
            tc.tile_pool(name="vbuf", bufs=1) as vpool,
            tc.tile_pool(name="ebuf", bufs=_B("E_BUFS", 2)) as epool,
            tc.tile_pool(name="dbuf", bufs=2) as dpool,
            tc.tile_pool(name="res", bufs=1) as rpool,
            tc.tile_pool(name="yout", bufs=4) as ypool,
            tc.tile_pool(name="ps_a", bufs=_B("PSA_BUFS", 2), space="PSUM") as ps_a,
            tc.tile_pool(name="ps_s", bufs=_B("PSS_BUFS", 2), space="PSUM") as ps_s,
            tc.tile_pool(name="ps_d", bufs=_B("PSD_BUFS", 2), space="PSUM") as ps_d,
            tc.tile_pool(name="ps_r", bufs=_B("PSR_BUFS", 2), space="PSUM") as ps_r,
        ):
            # ---- load weights (once) ----
            wqk_sb = wpool.tile([128, CT, GQK, DK], MMDT)
            nc.sync.dma_start(
                out=wqk_sb,
                in_=wqk.rearrange("(ct p) (g d) -> p ct g d", p=128, d=DK).bitcast(MMDT))
            wv_sb = wpool.tile([128, CT, C], MMDT)
            nc.sync.dma_start(
                out=wv_sb, in_=wv.rearrange("(ct p) f -> p ct f", p=128).bitcast(MMDT))
            wout_sb = wpool.tile([128, CT, C], MMDT)
            nc.sync.dma_start(
                out=wout_sb, in_=wout.rearrange("(ct p) f -> p ct f", p=128).bitcast(MMDT))
            bqk_sb = wpool.tile([128, GQK], dt.float32)
            nc.sync.dma_start(out=bqk_sb, in_=bqk)
            beff_sb = wpool.tile([128, CT], dt.float32)
            nc.sync.dma_start(out=beff_sb, in_=beff)
            ones_sb = wpool.tile([128, 128], MMDT)
            nc.sync.dma_start(out=ones_sb, in_=ones.bitcast(MMDT))

            for b in range(BPC):
                # ---- load x[b] as 4 stacked (128, N) channel tiles ----
                xT_sb = xpool.tile([128, CT, N], MMDT)
                nc.sync.dma_start(
                    out=xT_sb,
                    in_=x[b].rearrange("(ct p) n -> p ct n", p=128).bitcast(MMDT))

                # ---- q/k projection, transposed: qkT[g] = wqk[:,g]^T @ x ----
                qkT_sb = qkpool.tile([128, GQK, N], MMDT)
                for g in range(GQK):
                    for nb in range(NB):
                        ps = ps_a.tile([128, 512], dt.float32)
                        for ct in range(CT):
                            nc.tensor.matmul(
                                ps,
                                _mm(wqk_sb[:, ct, g, :]),
                                _mm(xT_sb[:, ct, bass.ts(nb, 512)]),
                                start=(ct == 0), stop=(ct == CT - 1))
                        nc.vector.tensor_scalar_add(
                            qkT_sb[:, g, bass.ts(nb, 512)], ps,
                            bqk_sb[:, g:g + 1])

                # ---- v projection, token-major: v[jt] = x[:,jt]^T @ wv ----
                v_sb = vpool.tile([128, JT, C], MMDT)
                for jt in range(JT):
                    ps = ps_a.tile([128, 512], dt.float32)
                    for ct in range(CT):
                        nc.tensor.matmul(
                            ps,
                            _mm(xT_sb[:, ct, bass.ts(jt, 128)]),
                            _mm(wv_sb[:, ct, :]),
                            start=(ct == 0), stop=(ct == CT - 1))
                    nc.vector.tensor_copy(v_sb[:, jt, :], ps)

                # ---- attention ----
                res_sb = rpool.tile([128, CT, N], MMDT)
                for h in range(HEADS):
                    for ib in range(NB):
                        # scores (j, i) then exp
                        eT = epool.tile([128, JT, 512], MMDT)
                        for jt in range(JT):
                            ps = ps_s.tile([128, 512], dt.float32)
                            nc.tensor.matmul(
                                ps,
                                _mm(qkT_sb[:, 2 * h + 1, bass.ts(jt, 128)]),
                                _mm(qkT_sb[:, 2 * h, bass.ts(ib, 512)]),
                                start=True, stop=True)
                            nc.scalar.activation(
                                eT[:, jt, :], ps,
                                mybir.ActivationFunctionType.Exp,
                                scale=SCALE)
                        # column sums D (replicated across partitions) and PV
                        psd = ps_d.tile([128, 512], dt.float32)
                        psr = ps_r.tile([128, 512], dt.float32)
                        for jt in range(JT):
                            nc.tensor.matmul(
                                psd, _mm(ones_sb), _mm(eT[:, jt, :]),
                                start=(jt == 0), stop=(jt == JT - 1))
                            nc.tensor.matmul(
                                psr,
                                _mm(v_sb[:, jt, bass.ts(h, DK)]),
                                _mm(eT[:, jt, :]),
                                start=(jt == 0), stop=(jt == JT - 1))
                        d_sb = dpool.tile([128, 512], dt.float32)
                        nc.vector.reciprocal(d_sb, psd)
                        nc.vector.tensor_mul(
                            res_sb[:, h, bass.ts(ib, 512)], psr, d_sb)

                # ---- output projection + bias + residual ----
                for cot in range(CT):
                    for nb in range(NB):
                        ps = ps_a.tile([128, 512], dt.float32)
                        for ct in range(CT):
                            nc.tensor.matmul(
                                ps,
                                _mm(wout_sb[:, ct, bass.ts(cot, 128)]),
                                _mm(res_sb[:, ct, bass.ts(nb, 512)]),
                                start=(ct == 0), stop=(ct == CT - 1))
                        y_sb = ypool.tile([128, 512], dt.float32)
                        nc.vector.scalar_tensor_tensor(
                            y_sb, ps, beff_sb[:, cot:cot + 1],
                            xT_sb[:, cot, bass.ts(nb, 512)].bitcast(dt.float32),
                            op0=mybir.AluOpType.add, op1=mybir.AluOpType.add)
                        nc.sync.dma_start(
                            out=y[b, bass.ts(cot, 128), bass.ts(nb, 512)],
                            in_=y_sb)
    nc.finalize()
    return nc


_CACHED_NC = None


def _get_program():
    global _CACHED_NC
    if _CACHED_NC is None:
        _CACHED_NC = build_program()
    return _CACHED_NC


def kernel(x, w_proj, b_proj, w_out, b_out):
    global LAST_RESULTS
    x = np.ascontiguousarray(np.asarray(x, dtype=np.float32)).reshape(B, C, N)
    w_proj = np.asarray(w_proj, dtype=np.float32)
    b_proj = np.asarray(b_proj, dtype=np.float32)
    w_out = np.asarray(w_out, dtype=np.float32)
    b_out = np.asarray(b_out, dtype=np.float32)

    # Host-side weight re-layout (no math on activations).
    w4 = w_proj.reshape(C, HEADS, 3, DK)
    wqk = np.ascontiguousarray(w4[:, :, :2, :].reshape(C, GQK * DK))
    wv = np.ascontiguousarray(w4[:, :, 2, :].reshape(C, C))
    b4 = b_proj.reshape(HEADS, 3, DK)
    bqk = np.ascontiguousarray(b4[:, :2, :].reshape(GQK, DK).T)
    # v-bias commutes through softmax-weighted averaging (rows sum to 1), so
    # it folds into the output bias: b_eff = b_out + b_v @ w_out.
    b_eff = b_out + b4[:, 2, :].reshape(C) @ w_out
    beff = np.ascontiguousarray(b_eff.reshape(CT, 128).T)

    nc = _get_program()
    in_maps = []
    for c in range(N_CORES):
        in_maps.append({
            "x": x[c * BPC:(c + 1) * BPC],
            "wqk": wqk,
            "wv": wv,
            "wout": np.ascontiguousarray(w_out),
            "bqk": bqk,
            "beff": beff,
            "ones": np.ones((128, 128), np.float32),
        })
    res = run_bass_kernel_spmd(nc, in_maps, list(range(N_CORES)))
    LAST_RESULTS = res
    out = np.concatenate([res.results[c]["y"] for c in range(N_CORES)], axis=0)
    return out.reshape(B, C, 32, 32)

